# revision 2
# baseline (speedup 1.0000x reference)
"""Trainium2 Bass kernel: 3-layer edge-weighted GraphSAGE (Cluster-GCN style).

Strategy (8 NeuronCores, SPMD):
  - Nodes padded to NPAD = 8*SHARD, shard k = rows [k*SHARD, (k+1)*SHARD).
  - Edges sorted by (dst tile, src half); per dst-tile (128 nodes) the
    incoming edges' src rows are gathered with dma_gather, then aggregated
    with a one-hot matmul into PSUM.
  - The one-hot selection matrices S[e, n] = (dst_e == n) * w'_e (with
    w' = edge_attr / max(indeg, 1), folding the mean) are identical for all
    three layers and are PRECOMPUTED ON THE HOST, uploaded to DRAM, and
    streamed into SBUF per chunk group -- this removes the per-chunk DVE
    tensor_scalar (is_equal*mult) ops that dominated the previous version
    (~455ns x 2646 ops = 1.2ms of DVE busy).
  - Layer outputs are AllGather'ed so every core holds the full feature
    table for the next layer's gather.  Layer 3 projects h2 @ Wl2 first
    (8 cols) so its gather moves 256B/edge instead of 1KB/edge.
  - dma_gather uses int16 indices, so tables are addressed as two halves
    (rows < HALF and >= HALF); gather calls are one per (tile-group, half)
    rotated across the 4 SWDGE queues.  Full chunks are always gathered
    (padding slots -> row 0) so no SBUF garbage (possible NaN) reaches the
    PSUM accumulation through the zero columns of S.
  - PSUM->SBUF copies are split between the Scalar engine (Act Copy only,
    so the activation table is loaded once) and the DVE; ReLU runs on the
    DVE (max with 0).  Layer 3's log_softmax skips the max-subtraction
    (|logits| is small) and batches: per-tile Exp with accumulate, a single
    Ln over all tiles at the end -- avoiding per-tile activation-table
    reloads (1.28us each).
  - bf16 matmul operands (fp32 PSUM accumulation) for full PE rate and
    half gather bandwidth; cfg.bf16=False falls back to all-fp32.
"""
import numpy as np

import concourse.bacc as bacc
import concourse.tile as tile
from concourse import mybir
from concourse.bass_utils import run_bass_kernel_spmd
from concourse.masks import make_identity

try:
    from ml_dtypes import bfloat16 as np_bf16
except ImportError:  # pragma: no cover
    import jax.numpy as _jnp
    np_bf16 = _jnp.bfloat16

F32 = mybir.dt.float32
BF16 = mybir.dt.bfloat16
I16 = mybir.dt.int16
P = 128
Alu = mybir.AluOpType
Act = mybir.ActivationFunctionType


class Cfg:
    def __init__(self, n_nodes=50000, n_edges=800000, dims=(128, 256, 256, 8),
                 ncores=8, G=4, bf16=True):
        self.N, self.E, self.D, self.NC = n_nodes, n_edges, dims, ncores
        self.SHARD = ((n_nodes + ncores * P - 1) // (ncores * P)) * P
        self.NPAD = self.SHARD * ncores
        self.TPC = self.SHARD // P
        self.HALF = self.NPAD // 2
        assert self.HALF < 32768 and self.NPAD % 2 == 0
        assert dims[0] == P and dims[1] % P == 0 and dims[2] % P == 0
        self.G, self.bf16 = G, bf16
        # L3 gather table row width (256B rows)
        self.EL3 = 128 if bf16 else 64

    def key(self):
        return (self.N, self.E, self.D, self.NC, self.G, self.bf16)


class Plan:
    def __init__(self, cfg, NAB):
        self.NAB = NAB
        self.groups = [list(range(i, min(cfg.TPC, i + cfg.G)))
                       for i in range(0, cfg.TPC, cfg.G)]
        self.cb = np.zeros((cfg.TPC, 2), np.int64)
        self.gbase, self.call_cb, self.gca, self.gcb = [], [], [], []
        self.group_of = np.zeros(cfg.TPC, np.int64)
        c = 0
        for gi, tl in enumerate(self.groups):
            self.gbase.append(c)
            cbA = c
            for t in tl:
                self.group_of[t] = gi
                self.cb[t, 0] = c
                c += NAB[t, 0]
            cbB = c
            for t in tl:
                self.cb[t, 1] = c
                c += NAB[t, 1]
            self.call_cb.append((cbA, cbB))
            self.gca.append(cbB - cbA)
            self.gcb.append(c - cbB)
        self.CT = c


def host_prep(cfg, x, edge_index, edge_attr):
    src = edge_index[0].astype(np.int64)
    dst = edge_index[1].astype(np.int64)
    cnt = np.bincount(dst, minlength=cfg.N).astype(np.float32)
    wp = (edge_attr.astype(np.float32)
          / np.maximum(cnt, 1.0)[dst]).astype(np.float32)

    segkey = (dst >> 7) * 2 + (src >= cfg.HALF)
    order = np.argsort(segkey, kind="stable")
    ssrc, sdst, swp = src[order], dst[order], wp[order]
    nseg = (cfg.NPAD // P) * 2
    seg_counts = np.bincount(segkey, minlength=nseg)
    seg_start = np.zeros(nseg + 1, np.int64)
    seg_start[1:] = np.cumsum(seg_counts)
    sc = seg_counts.reshape(cfg.NC, cfg.TPC, 2)
    NAB = np.maximum(
        np.ceil(sc / P).astype(np.int64).max(axis=0), 1)  # [TPC, 2]
    plan = Plan(cfg, NAB)
    CT = plan.CT

    idx_arr = np.zeros((cfg.NC, 16, CT * 8), np.int16)
    # one-hot selection matrices, chunk-major: s_arr[k, e, c, n]
    s_arr = np.zeros((cfg.NC, P, CT, P), np.float32)
    for k in range(cfg.NC):
        for t in range(cfg.TPC):
            for h in (0, 1):
                si = (k * cfg.TPC + t) * 2 + h
                i0, n = seg_start[si], seg_counts[si]
                if n == 0:
                    continue
                rows = (ssrc[i0:i0 + n] - h * cfg.HALF).astype(np.int16)
                stbase = k * cfg.SHARD + t * P
                dl = (sdst[i0:i0 + n] - stbase).astype(np.int64)
                cb = plan.cb[t, h]
                j = np.arange(n)
                s_arr[k, j % P, cb + j // P, dl] = swp[i0:i0 + n]
                ccb = plan.call_cb[plan.group_of[t]][h]
                ic = (cb - ccb) * P + j
                idx_arr[k, ic % 16, ccb * 8 + ic // 16] = rows
    idx_full = np.tile(idx_arr, (1, 8, 1))
    return plan, idx_full, s_arr


def build_nc(cfg, plan):
    d0, d1, d2, d3 = cfg.D
    H1, H2 = d1 // P, d2 // P
    DT = BF16 if cfg.bf16 else F32
    CT = plan.CT
    NAB = plan.NAB

    nc = bacc.Bacc("TRN2", target_bir_lowering=False, debug=False,
                   num_devices=cfg.NC, enable_asserts=False,
                   num_swdge_queues=4)

    xfull = nc.dram_tensor("xfull", [cfg.NPAD, d0], DT, kind="ExternalInput")
    xT_in = nc.dram_tensor("xT", [P, cfg.SHARD], DT, kind="ExternalInput")
    idx_in = nc.dram_tensor("idx", [P, CT * 8], I16, kind="ExternalInput")
    sfull_in = nc.dram_tensor("sfull", [P, CT, P], DT, kind="ExternalInput")
    wl0_in = nc.dram_tensor("wl0", [P, d1], DT, kind="ExternalInput")
    wr0_in = nc.dram_tensor("wr0", [P, d1], DT, kind="ExternalInput")
    wl1_in = nc.dram_tensor("wl1", [P, H1, d2], DT, kind="ExternalInput")
    wr1_in = nc.dram_tensor("wr1", [P, H1, d2], DT, kind="ExternalInput")
    wl2_in = nc.dram_tensor("wl2", [P, H2, d3], DT, kind="ExternalInput")
    wr2_in = nc.dram_tensor("wr2", [P, H2, d3], DT, kind="ExternalInput")
    b0_in = nc.dram_tensor("b0", [1, d1], DT, kind="ExternalInput")
    b1_in = nc.dram_tensor("b1", [1, d2], DT, kind="ExternalInput")
    b2_in = nc.dram_tensor("b2", [1, d3], DT, kind="ExternalInput")
    out_t = nc.dram_tensor("out", [cfg.SHARD, d3], F32, kind="ExternalOutput")

    with tile.TileContext(nc) as tc:
        with (
            tc.tile_pool(name="const", bufs=1) as cp,
            tc.tile_pool(name="mt", bufs=2) as mp,
            tc.tile_pool(name="st", bufs=2) as sp,
            tc.tile_pool(name="wk", bufs=2) as wk,
            tc.tile_pool(name="psum", bufs=2, space="PSUM") as ps,
            tc.tile_pool(name="dram", bufs=1, space="DRAM") as dr,
        ):
            h1own = dr.tile([cfg.SHARD, d1], DT)
            h1full = dr.tile([cfg.NPAD, d1], DT, addr_space="Shared")
            h1T = dr.tile([d1, cfg.SHARD], DT)
            h2T = dr.tile([d2, cfg.SHARD], DT)
            h2p = dr.tile([cfg.SHARD, cfg.EL3], DT)
            h2pfull = dr.tile([cfg.NPAD, cfg.EL3], DT, addr_space="Shared")

            # ---- constants / parameters
            ident_f = cp.tile([P, P], F32)
            make_identity(nc, ident_f[:])
            if cfg.bf16:
                ident_b = cp.tile([P, P], BF16)
                nc.vector.tensor_copy(ident_b[:], ident_f[:])
                ident_dt = ident_b
            else:
                ident_dt = ident_f
            ones_t = cp.tile([1, P], DT)
            nc.vector.memset(ones_t[:], 1.0)
            xT_t = cp.tile([P, cfg.SHARD], DT)
            nc.sync.dma_start(out=xT_t[:], in_=xT_in[:, :])
            idx_t = cp.tile([P, CT * 8], I16)
            nc.sync.dma_start(out=idx_t[:], in_=idx_in[:, :])
            wl0_t = cp.tile([P, d1], DT)
            nc.sync.dma_start(out=wl0_t[:], in_=wl0_in[:, :])
            wr0_t = cp.tile([P, d1], DT)
            nc.sync.dma_start(out=wr0_t[:], in_=wr0_in[:, :])
            wl1_t = cp.tile([P, H1, d2], DT)
            nc.sync.dma_start(out=wl1_t[:], in_=wl1_in[:, :, :])
            wr1_t = cp.tile([P, H1, d2], DT)
            nc.sync.dma_start(out=wr1_t[:], in_=wr1_in[:, :, :])
            wl2_t = cp.tile([P, H2, d3], DT)
            nc.sync.dma_start(out=wl2_t[:], in_=wl2_in[:, :, :])
            wr2_t = cp.tile([P, H2, d3], DT)
            nc.sync.dma_start(out=wr2_t[:], in_=wr2_in[:, :, :])
            b0_t = cp.tile([1, d1], DT)
            nc.sync.dma_start(out=b0_t[:], in_=b0_in[:, :])
            b1_t = cp.tile([1, d2], DT)
            nc.sync.dma_start(out=b1_t[:], in_=b1_in[:, :])
            b2_t = cp.tile([1, d3], DT)
            nc.sync.dma_start(out=b2_t[:], in_=b2_in[:, :])

            qctr = [0]  # round-robin SWDGE queue counter

            def gathers(gi, table, elem):
                # one gather per (group, half), rotated across the 4 SWDGE
                # queues; plus the DMA stream of this group's S chunks
                cgA, cgB = plan.gca[gi], plan.gcb[gi]
                gb = plan.gbase[gi]
                m_t = mp.tile([P, cgA + cgB, elem], DT, tag="mt")
                s_t = sp.tile([P, cgA + cgB, P], DT, tag="st")
                nc.sync.dma_start(out=s_t[:],
                                  in_=sfull_in[:, gb:gb + cgA + cgB, :])
                for tb, cb0, gc in ((table[0:cfg.HALF, :],
                                     plan.call_cb[gi][0], cgA),
                                    (table[cfg.HALF:cfg.NPAD, :],
                                     plan.call_cb[gi][1], cgB)):
                    nidx = gc * P
                    nc.gpsimd.dma_gather(
                        m_t[:, cb0 - gb:cb0 - gb + gc, :], tb,
                        idx_t[:, cb0 * 8:cb0 * 8 + gc * 8],
                        nidx, nidx, elem, single_packet=False,
                        queue_num=qctr[0] % 4)
                    qctr[0] += 1
                return m_t, s_t

            def tile_chunks(t):
                return (list(range(plan.cb[t, 0], plan.cb[t, 0] + NAB[t, 0]))
                        + list(range(plan.cb[t, 1], plan.cb[t, 1] + NAB[t, 1])))

            # ---------------- Layer 1 ----------------
            for gi, tl in enumerate(plan.groups):
                m_t, s_t = gathers(gi, xfull, d0)
                gb = plan.gbase[gi]
                for t in tl:
                    aggT = ps.tile([P, P], F32, tag="agg")
                    chunks = tile_chunks(t)
                    for ci, c in enumerate(chunks):
                        nc.tensor.matmul(
                            out=aggT[:], lhsT=m_t[:, c - gb, :],
                            rhs=s_t[:, c - gb, :],
                            start=(ci == 0), stop=(ci == len(chunks) - 1))
                    meanT = wk.tile([P, P], DT, tag="meanT")
                    nc.scalar.activation(meanT[:], aggT[:], Act.Copy)
                    op_ = ps.tile([P, d1], F32, tag="outp")
                    nc.tensor.matmul(out=op_[:], lhsT=meanT[:],
                                     rhs=wl0_t[:], start=True, stop=False)
                    nc.tensor.matmul(out=op_[:],
                                     lhsT=xT_t[:, t * P:(t + 1) * P],
                                     rhs=wr0_t[:], start=False, stop=False)
                    nc.tensor.matmul(out=op_[:], lhsT=ones_t[:],
                                     rhs=b0_t[:], start=False,
                                     stop=True, skip_group_check=True)
                    h_sb = wk.tile([P, d1], DT, tag="h_sb")
                    nc.vector.tensor_scalar(out=h_sb[:], in0=op_[:],
                                            scalar1=0.0, scalar2=None,
                                            op0=Alu.max)  # relu + cast
                    nc.sync.dma_start(out=h1own[t * P:(t + 1) * P, :],
                                      in_=h_sb[:])
                    for hh in range(H1):
                        trp = ps.tile([P, P], DT, tag="trp")
                        nc.tensor.transpose(
                            out=trp[:], in_=h_sb[:, hh * P:(hh + 1) * P],
                            identity=ident_dt[:])
                        hT_sb = wk.tile([P, P], DT, tag="hT_sb")
                        nc.vector.tensor_copy(hT_sb[:], trp[:])
                        nc.sync.dma_start(
                            out=h1T[hh * P:(hh + 1) * P, t * P:(t + 1) * P],
                            in_=hT_sb[:])

            nc.gpsimd.collective_compute(
                "AllGather", Alu.bypass,
                replica_groups=[list(range(cfg.NC))],
                ins=[h1own.opt()], outs=[h1full.opt()])

            # ---------------- Layer 2 ----------------
            for gi, tl in enumerate(plan.groups):
                m_t, s_t = gathers(gi, h1full, d1)
                gb = plan.gbase[gi]
                for t in tl:
                    agg = ps.tile([P, d1], F32, tag="outp")
                    chunks = tile_chunks(t)
                    for ci, c in enumerate(chunks):
                        nc.tensor.matmul(
                            out=agg[:], lhsT=s_t[:, c - gb, :],
                            rhs=m_t[:, c - gb, :],
                            start=(ci == 0), stop=(ci == len(chunks) - 1))
                    agg_sb = wk.tile([P, d1], DT, tag="agg_sb")
                    nc.scalar.activation(agg_sb[:], agg[:], Act.Copy)
                    mts = []
                    for hh in range(H1):
                        trp = ps.tile([P, P], DT, tag="trp")
                        nc.tensor.transpose(
                            out=trp[:], in_=agg_sb[:, hh * P:(hh + 1) * P],
                            identity=ident_dt[:])
                        mt_sb = wk.tile([P, P], DT, tag="mT2")
                        nc.scalar.activation(mt_sb[:], trp[:], Act.Copy)
                        mts.append(mt_sb)
                    h1T_t = wk.tile([P, H1, P], DT, tag="hTt")
                    for hh in range(H1):
                        nc.sync.dma_start(
                            out=h1T_t[:, hh, :],
                            in_=h1T[hh * P:(hh + 1) * P, t * P:(t + 1) * P])
                    op_ = ps.tile([P, d2], F32, tag="outp")
                    for hh in range(H1):
                        nc.tensor.matmul(out=op_[:], lhsT=mts[hh][:],
                                         rhs=wl1_t[:, hh, :],
                                         start=(hh == 0), stop=False)
                    for hh in range(H1):
                        nc.tensor.matmul(out=op_[:], lhsT=h1T_t[:, hh, :],
                                         rhs=wr1_t[:, hh, :],
                                         start=False, stop=False)
                    nc.tensor.matmul(out=op_[:], lhsT=ones_t[:],
                                     rhs=b1_t[:], start=False, stop=True,
                                     skip_group_check=True)
                    h_sb = wk.tile([P, d2], DT, tag="h_sb")
                    nc.vector.tensor_scalar(out=h_sb[:], in0=op_[:],
                                            scalar1=0.0, scalar2=None,
                                            op0=Alu.max)  # relu + cast
                    hts = []
                    for hh in range(H2):
                        trp = ps.tile([P, P], DT, tag="trp")
                        nc.tensor.transpose(
                            out=trp[:], in_=h_sb[:, hh * P:(hh + 1) * P],
                            identity=ident_dt[:])
                        hT_sb = wk.tile([P, P], DT, tag="hT_sb")
                        nc.vector.tensor_copy(hT_sb[:], trp[:])
                        nc.sync.dma_start(
                            out=h2T[hh * P:(hh + 1) * P, t * P:(t + 1) * P],
                            in_=hT_sb[:])
                        hts.append(hT_sb)
                    prj = ps.tile([P, d3], F32, tag="proj")
                    for hh in range(H2):
                        nc.tensor.matmul(out=prj[:], lhsT=hts[hh][:],
                                         rhs=wl2_t[:, hh, :],
                                         start=(hh == 0), stop=(hh == H2 - 1))
                    prj_sb = wk.tile([P, d3], DT, tag="prj_sb")
                    nc.vector.tensor_copy(prj_sb[:], prj[:])
                    nc.sync.dma_start(out=h2p[t * P:(t + 1) * P, 0:d3],
                                      in_=prj_sb[:])

            nc.gpsimd.collective_compute(
                "AllGather", Alu.bypass,
                replica_groups=[list(range(cfg.NC))],
                ins=[h2p.opt()], outs=[h2pfull.opt()])

            # ---------------- Layer 3 ----------------
            # log_softmax without max-subtraction (logit scale is small):
            # z - ln(sum(exp(z))); Exp accumulates per tile, one Ln at the
            # end (avoids per-tile activation-table reloads)
            se_all = cp.tile([P, cfg.TPC], F32)
            z_all = cp.tile([P, cfg.TPC, d3], F32)
            for gi, tl in enumerate(plan.groups):
                m_t, s_t = gathers(gi, h2pfull, cfg.EL3)
                gb = plan.gbase[gi]
                for t in tl:
                    op_ = ps.tile([P, d3], F32, tag="proj")
                    chunks = tile_chunks(t)
                    for ci, c in enumerate(chunks):
                        nc.tensor.matmul(
                            out=op_[:], lhsT=s_t[:, c - gb, :],
                            rhs=m_t[:, c - gb, 0:d3],
                            start=(ci == 0), stop=False)
                    h2T_t = wk.tile([P, H2, P], DT, tag="hTt")
                    for hh in range(H2):
                        nc.sync.dma_start(
                            out=h2T_t[:, hh, :],
                            in_=h2T[hh * P:(hh + 1) * P, t * P:(t + 1) * P])
                    for hh in range(H2):
                        nc.tensor.matmul(out=op_[:], lhsT=h2T_t[:, hh, :],
                                         rhs=wr2_t[:, hh, :],
                                         start=False, stop=False,
                                         skip_group_check=True)
                    nc.tensor.matmul(out=op_[:], lhsT=ones_t[:],
                                     rhs=b2_t[:], start=False, stop=True,
                                     skip_group_check=True)
                    nc.vector.tensor_copy(z_all[:, t, :], op_[:])
                    e_dummy = wk.tile([P, d3], F32, tag="e_sb")
                    nc.scalar.activation(e_dummy[:], op_[:], Act.Exp,
                                         accum_out=se_all[:, t:t + 1])
            ls_all = cp.tile([P, cfg.TPC], F32)
            nc.scalar.activation(ls_all[:], se_all[:], Act.Ln)
            for t in range(cfg.TPC):
                out_sb = wk.tile([P, d3], F32, tag="out_sb")
                nc.vector.tensor_scalar(out=out_sb[:], in0=z_all[:, t, :],
                                        scalar1=ls_all[:, t:t + 1],
                                        scalar2=None, op0=Alu.subtract)
                nc.sync.dma_start(out=out_t[t * P:(t + 1) * P, :],
                                  in_=out_sb[:])

    nc.compile()
    return nc


_NC_CACHE = {}


def get_nc(cfg, plan):
    key = (cfg.key(), plan.NAB.tobytes())
    if key not in _NC_CACHE:
        _NC_CACHE[key] = build_nc(cfg, plan)
    return _NC_CACHE[key]


def run(cfg, inputs, trace=False, tmpdir=None):
    x = np.asarray(inputs["x"], np.float32)
    plan, idx_full, s_arr = host_prep(
        cfg, x, np.asarray(inputs["edge_index"]),
        np.asarray(inputs["edge_attr"], np.float32))
    d0, d1, d2, d3 = cfg.D
    H1, H2 = d1 // P, d2 // P
    npDT = np_bf16 if cfg.bf16 else np.float32

    xpad = np.zeros((cfg.NPAD, d0), np.float32)
    xpad[:cfg.N] = x
    xpad = xpad.astype(npDT)
    Wl1 = np.asarray(inputs["Wl1"], np.float32)
    Wr1 = np.asarray(inputs["Wr1"], np.float32)
    Wl2 = np.asarray(inputs["Wl2"], np.float32)
    Wr2 = np.asarray(inputs["Wr2"], np.float32)
    shared = {
        "xfull": xpad,
        "wl0": np.asarray(inputs["Wl0"], np.float32).astype(npDT),
        "wr0": np.asarray(inputs["Wr0"], np.float32).astype(npDT),
        "wl1": Wl1.reshape(H1, P, d2).transpose(1, 0, 2).astype(npDT),
        "wr1": Wr1.reshape(H1, P, d2).transpose(1, 0, 2).astype(npDT),
        "wl2": Wl2.reshape(H2, P, d3).transpose(1, 0, 2).astype(npDT),
        "wr2": Wr2.reshape(H2, P, d3).transpose(1, 0, 2).astype(npDT),
        "b0": (np.asarray(inputs["bl0"]) + np.asarray(inputs["br0"]))
        .astype(np.float32)[None, :].astype(npDT),
        "b1": (np.asarray(inputs["bl1"]) + np.asarray(inputs["br1"]))
        .astype(np.float32)[None, :].astype(npDT),
        "b2": (np.asarray(inputs["bl2"]) + np.asarray(inputs["br2"]))
        .astype(np.float32)[None, :].astype(npDT),
    }
    in_maps = []
    for k in range(cfg.NC):
        in_maps.append({
            **shared,
            "xT": np.ascontiguousarray(
                xpad[k * cfg.SHARD:(k + 1) * cfg.SHARD].T),
            "idx": idx_full[k],
            "sfull": s_arr[k].astype(npDT),
        })
    nc = get_nc(cfg, plan)
    res = run_bass_kernel_spmd(nc, in_maps, core_ids=list(range(cfg.NC)),
                               trace=trace, tmpdir=tmpdir)
    out = np.concatenate([res.results[k]["out"] for k in range(cfg.NC)],
                         axis=0)[:cfg.N]
    return np.ascontiguousarray(out.astype(np.float32)), res


def kernel(**inputs):
    cfg = Cfg()
    out, _ = run(cfg, inputs)
    return out


# revision 5
# speedup vs baseline: 1.1249x; 1.1249x over previous
"""Trainium2 Bass kernel: 3-layer edge-weighted GraphSAGE (Cluster-GCN style).

Strategy (8 NeuronCores, SPMD):
  - Nodes padded to NPAD = 8*SHARD, shard k = rows [k*SHARD, (k+1)*SHARD).
  - Edges sorted by (dst tile, src half); per dst-tile (128 nodes) the
    incoming edges' src rows are gathered with dma_gather, then aggregated
    with a one-hot matmul into PSUM.
  - The one-hot selection matrices S[e, n] = (dst_e == n) * w'_e (with
    w' = edge_attr / max(indeg, 1), folding the mean) are identical for all
    three layers and are PRECOMPUTED ON THE HOST, uploaded to DRAM, and
    streamed into SBUF per chunk group -- this removes the per-chunk DVE
    tensor_scalar (is_equal*mult) ops that dominated the previous version
    (~455ns x 2646 ops = 1.2ms of DVE busy).
  - Layer outputs are AllGather'ed so every core holds the full feature
    table for the next layer's gather.  Layer 3 projects h2 @ Wl2 first
    (8 cols) so its gather moves 256B/edge instead of 1KB/edge.
  - dma_gather uses int16 indices, so tables are addressed as two halves
    (rows < HALF and >= HALF); gather calls are one per (tile-group, half)
    rotated across the 4 SWDGE queues.  Full chunks are always gathered
    (padding slots -> row 0) so no SBUF garbage (possible NaN) reaches the
    PSUM accumulation through the zero columns of S.
  - PSUM->SBUF copies are split between the Scalar engine (Act Copy only,
    so the activation table is loaded once) and the DVE; ReLU runs on the
    DVE (max with 0).  Layer 3's log_softmax skips the max-subtraction
    (|logits| is small) and batches: per-tile Exp with accumulate, a single
    Ln over all tiles at the end -- avoiding per-tile activation-table
    reloads (1.28us each).
  - bf16 matmul operands (fp32 PSUM accumulation) for full PE rate and
    half gather bandwidth; cfg.bf16=False falls back to all-fp32.
"""
import numpy as np

import concourse.bacc as bacc
import concourse.tile as tile
from concourse import mybir
from concourse.bass_utils import run_bass_kernel_spmd
from concourse.masks import make_identity

try:
    from ml_dtypes import bfloat16 as np_bf16
except ImportError:  # pragma: no cover
    import jax.numpy as _jnp
    np_bf16 = _jnp.bfloat16

F32 = mybir.dt.float32
BF16 = mybir.dt.bfloat16
I16 = mybir.dt.int16
P = 128
Alu = mybir.AluOpType
Act = mybir.ActivationFunctionType


class Cfg:
    def __init__(self, n_nodes=50000, n_edges=800000, dims=(128, 256, 256, 8),
                 ncores=8, G=2, bf16=True):
        self.N, self.E, self.D, self.NC = n_nodes, n_edges, dims, ncores
        self.SHARD = ((n_nodes + ncores * P - 1) // (ncores * P)) * P
        self.NPAD = self.SHARD * ncores
        self.TPC = self.SHARD // P
        self.HALF = self.NPAD // 2
        assert self.HALF < 32768 and self.NPAD % 2 == 0
        assert dims[0] == P and dims[1] % P == 0 and dims[2] % P == 0
        self.G, self.bf16 = G, bf16
        # L3 gather table row width (256B rows)
        self.EL3 = 128 if bf16 else 64

    def key(self):
        return (self.N, self.E, self.D, self.NC, self.G, self.bf16)


class Plan:
    def __init__(self, cfg, NAB):
        self.NAB = NAB
        self.groups = [list(range(i, min(cfg.TPC, i + cfg.G)))
                       for i in range(0, cfg.TPC, cfg.G)]
        self.cb = np.zeros((cfg.TPC, 2), np.int64)
        self.gbase, self.call_cb, self.gca, self.gcb = [], [], [], []
        self.group_of = np.zeros(cfg.TPC, np.int64)
        c = 0
        for gi, tl in enumerate(self.groups):
            self.gbase.append(c)
            cbA = c
            for t in tl:
                self.group_of[t] = gi
                self.cb[t, 0] = c
                c += NAB[t, 0]
            cbB = c
            for t in tl:
                self.cb[t, 1] = c
                c += NAB[t, 1]
            self.call_cb.append((cbA, cbB))
            self.gca.append(cbB - cbA)
            self.gcb.append(c - cbB)
        self.CT = c


def host_prep(cfg, x, edge_index, edge_attr):
    src = edge_index[0].astype(np.int64)
    dst = edge_index[1].astype(np.int64)
    cnt = np.bincount(dst, minlength=cfg.N).astype(np.float32)
    wp = (edge_attr.astype(np.float32)
          / np.maximum(cnt, 1.0)[dst]).astype(np.float32)

    segkey = (dst >> 7) * 2 + (src >= cfg.HALF)
    order = np.argsort(segkey, kind="stable")
    ssrc, sdst, swp = src[order], dst[order], wp[order]
    nseg = (cfg.NPAD // P) * 2
    seg_counts = np.bincount(segkey, minlength=nseg)
    seg_start = np.zeros(nseg + 1, np.int64)
    seg_start[1:] = np.cumsum(seg_counts)
    sc = seg_counts.reshape(cfg.NC, cfg.TPC, 2)
    NAB = np.maximum(
        np.ceil(sc / P).astype(np.int64).max(axis=0), 1)  # [TPC, 2]
    plan = Plan(cfg, NAB)
    CT = plan.CT

    idx_arr = np.zeros((cfg.NC, 16, CT * 8), np.int16)
    # one-hot selection matrices, chunk-major: s_arr[k, e, c, n]
    s_arr = np.zeros((cfg.NC, P, CT, P), np.float32)
    for k in range(cfg.NC):
        for t in range(cfg.TPC):
            for h in (0, 1):
                si = (k * cfg.TPC + t) * 2 + h
                i0, n = seg_start[si], seg_counts[si]
                if n == 0:
                    continue
                rows = (ssrc[i0:i0 + n] - h * cfg.HALF).astype(np.int16)
                stbase = k * cfg.SHARD + t * P
                dl = (sdst[i0:i0 + n] - stbase).astype(np.int64)
                cb = plan.cb[t, h]
                j = np.arange(n)
                s_arr[k, j % P, cb + j // P, dl] = swp[i0:i0 + n]
                ccb = plan.call_cb[plan.group_of[t]][h]
                ic = (cb - ccb) * P + j
                idx_arr[k, ic % 16, ccb * 8 + ic // 16] = rows
    idx_full = np.tile(idx_arr, (1, 8, 1))
    return plan, idx_full, s_arr


def build_nc(cfg, plan):
    d0, d1, d2, d3 = cfg.D
    H1, H2 = d1 // P, d2 // P
    DT = BF16 if cfg.bf16 else F32
    CT = plan.CT
    NAB = plan.NAB

    nc = bacc.Bacc("TRN2", target_bir_lowering=False, debug=False,
                   num_devices=cfg.NC, enable_asserts=False,
                   num_swdge_queues=4)

    xfull = nc.dram_tensor("xfull", [cfg.NPAD, d0], DT, kind="ExternalInput")
    xT_in = nc.dram_tensor("xT", [P, cfg.SHARD], DT, kind="ExternalInput")
    idx_in = nc.dram_tensor("idx", [P, CT * 8], I16, kind="ExternalInput")
    sfull_in = nc.dram_tensor("sfull", [P, CT, P], DT, kind="ExternalInput")
    wl0_in = nc.dram_tensor("wl0", [P, d1], DT, kind="ExternalInput")
    wr0_in = nc.dram_tensor("wr0", [P, d1], DT, kind="ExternalInput")
    wl1_in = nc.dram_tensor("wl1", [P, H1, d2], DT, kind="ExternalInput")
    wr1_in = nc.dram_tensor("wr1", [P, H1, d2], DT, kind="ExternalInput")
    wl2_in = nc.dram_tensor("wl2", [P, H2, d3], DT, kind="ExternalInput")
    wr2_in = nc.dram_tensor("wr2", [P, H2, d3], DT, kind="ExternalInput")
    b0_in = nc.dram_tensor("b0", [1, d1], DT, kind="ExternalInput")
    b1_in = nc.dram_tensor("b1", [1, d2], DT, kind="ExternalInput")
    b2_in = nc.dram_tensor("b2", [1, d3], DT, kind="ExternalInput")
    out_t = nc.dram_tensor("out", [cfg.SHARD, d3], F32, kind="ExternalOutput")

    with tile.TileContext(nc) as tc:
        with (
            tc.tile_pool(name="const", bufs=1) as cp,
            tc.tile_pool(name="mt", bufs=3) as mp,
            tc.tile_pool(name="st", bufs=3) as sp,
            tc.tile_pool(name="wk", bufs=2) as wk,
            tc.tile_pool(name="psum", bufs=2, space="PSUM") as ps,
            tc.tile_pool(name="dram", bufs=1, space="DRAM") as dr,
        ):
            h1own = dr.tile([cfg.SHARD, d1], DT)
            h1full = dr.tile([cfg.NPAD, d1], DT, addr_space="Shared")
            h1T = dr.tile([d1, cfg.SHARD], DT)
            h2T = dr.tile([d2, cfg.SHARD], DT)
            h2p = dr.tile([cfg.SHARD, cfg.EL3], DT)
            h2pfull = dr.tile([cfg.NPAD, cfg.EL3], DT, addr_space="Shared")

            # ---- constants / parameters
            ident_f = cp.tile([P, P], F32)
            make_identity(nc, ident_f[:])
            if cfg.bf16:
                ident_b = cp.tile([P, P], BF16)
                nc.vector.tensor_copy(ident_b[:], ident_f[:])
                ident_dt = ident_b
            else:
                ident_dt = ident_f
            ones_t = cp.tile([1, P], DT)
            nc.vector.memset(ones_t[:], 1.0)
            xT_t = cp.tile([P, cfg.SHARD], DT)
            nc.sync.dma_start(out=xT_t[:], in_=xT_in[:, :])
            idx_t = cp.tile([P, CT * 8], I16)
            nc.sync.dma_start(out=idx_t[:], in_=idx_in[:, :])
            wl0_t = cp.tile([P, d1], DT)
            nc.sync.dma_start(out=wl0_t[:], in_=wl0_in[:, :])
            wr0_t = cp.tile([P, d1], DT)
            nc.sync.dma_start(out=wr0_t[:], in_=wr0_in[:, :])
            wl1_t = cp.tile([P, H1, d2], DT)
            nc.sync.dma_start(out=wl1_t[:], in_=wl1_in[:, :, :])
            wr1_t = cp.tile([P, H1, d2], DT)
            nc.sync.dma_start(out=wr1_t[:], in_=wr1_in[:, :, :])
            wl2_t = cp.tile([P, H2, d3], DT)
            nc.sync.dma_start(out=wl2_t[:], in_=wl2_in[:, :, :])
            wr2_t = cp.tile([P, H2, d3], DT)
            nc.sync.dma_start(out=wr2_t[:], in_=wr2_in[:, :, :])
            b0_t = cp.tile([1, d1], DT)
            nc.sync.dma_start(out=b0_t[:], in_=b0_in[:, :])
            b1_t = cp.tile([1, d2], DT)
            nc.sync.dma_start(out=b1_t[:], in_=b1_in[:, :])
            b2_t = cp.tile([1, d3], DT)
            nc.sync.dma_start(out=b2_t[:], in_=b2_in[:, :])

            qctr = [0]  # round-robin SWDGE queue counter

            def gathers(gi, table, elem):
                # one gather per (tile, half), rotated across the 4 SWDGE
                # queues so descriptor generation runs concurrently on the
                # GpSimd Q7 lanes; plus the DMA stream of the S chunks
                cgA, cgB = plan.gca[gi], plan.gcb[gi]
                gb = plan.gbase[gi]
                m_t = mp.tile([P, cgA + cgB, elem], DT, tag="mt")
                s_t = sp.tile([P, cgA + cgB, P], DT, tag="st")
                nc.sync.dma_start(out=s_t[:],
                                  in_=sfull_in[:, gb:gb + cgA + cgB, :])
                for h, tb in ((0, table[0:cfg.HALF, :]),
                              (1, table[cfg.HALF:cfg.NPAD, :])):
                    for t in plan.groups[gi]:
                        nch = int(NAB[t, h])
                        nidx = nch * P
                        cb0 = plan.cb[t, h]
                        nc.gpsimd.dma_gather(
                            m_t[:, cb0 - gb:cb0 - gb + nch, :], tb,
                            idx_t[:, cb0 * 8:cb0 * 8 + nch * 8],
                            nidx, nidx, elem, single_packet=False,
                            queue_num=qctr[0] % 4)
                        qctr[0] += 1
                return m_t, s_t

            def tile_chunks(t):
                return (list(range(plan.cb[t, 0], plan.cb[t, 0] + NAB[t, 0]))
                        + list(range(plan.cb[t, 1], plan.cb[t, 1] + NAB[t, 1])))

            # ---------------- Layer 1 ----------------
            for gi, tl in enumerate(plan.groups):
                m_t, s_t = gathers(gi, xfull, d0)
                gb = plan.gbase[gi]
                for t in tl:
                    aggT = ps.tile([P, P], F32, tag="agg")
                    chunks = tile_chunks(t)
                    for ci, c in enumerate(chunks):
                        nc.tensor.matmul(
                            out=aggT[:], lhsT=m_t[:, c - gb, :],
                            rhs=s_t[:, c - gb, :],
                            start=(ci == 0), stop=(ci == len(chunks) - 1))
                    meanT = wk.tile([P, P], DT, tag="meanT")
                    nc.scalar.activation(meanT[:], aggT[:], Act.Copy)
                    op_ = ps.tile([P, d1], F32, tag="outp")
                    nc.tensor.matmul(out=op_[:], lhsT=meanT[:],
                                     rhs=wl0_t[:], start=True, stop=False)
                    nc.tensor.matmul(out=op_[:],
                                     lhsT=xT_t[:, t * P:(t + 1) * P],
                                     rhs=wr0_t[:], start=False, stop=False)
                    nc.tensor.matmul(out=op_[:], lhsT=ones_t[:],
                                     rhs=b0_t[:], start=False,
                                     stop=True, skip_group_check=True)
                    h_sb = wk.tile([P, d1], DT, tag="h_sb")
                    nc.vector.tensor_scalar(out=h_sb[:], in0=op_[:],
                                            scalar1=0.0, scalar2=None,
                                            op0=Alu.max)  # relu + cast
                    nc.sync.dma_start(out=h1own[t * P:(t + 1) * P, :],
                                      in_=h_sb[:])
                    for hh in range(H1):
                        trp = ps.tile([P, P], DT, tag="trp")
                        nc.tensor.transpose(
                            out=trp[:], in_=h_sb[:, hh * P:(hh + 1) * P],
                            identity=ident_dt[:])
                        hT_sb = wk.tile([P, P], DT, tag="hT_sb")
                        nc.vector.tensor_copy(hT_sb[:], trp[:])
                        nc.sync.dma_start(
                            out=h1T[hh * P:(hh + 1) * P, t * P:(t + 1) * P],
                            in_=hT_sb[:])

            nc.gpsimd.collective_compute(
                "AllGather", Alu.bypass,
                replica_groups=[list(range(cfg.NC))],
                ins=[h1own.opt()], outs=[h1full.opt()])

            # ---------------- Layer 2 ----------------
            for gi, tl in enumerate(plan.groups):
                m_t, s_t = gathers(gi, h1full, d1)
                gb = plan.gbase[gi]
                for t in tl:
                    agg = ps.tile([P, d1], F32, tag="outp")
                    chunks = tile_chunks(t)
                    for ci, c in enumerate(chunks):
                        nc.tensor.matmul(
                            out=agg[:], lhsT=s_t[:, c - gb, :],
                            rhs=m_t[:, c - gb, :],
                            start=(ci == 0), stop=(ci == len(chunks) - 1))
                    agg_sb = wk.tile([P, d1], DT, tag="agg_sb")
                    nc.scalar.activation(agg_sb[:], agg[:], Act.Copy)
                    mts = []
                    for hh in range(H1):
                        trp = ps.tile([P, P], DT, tag="trp")
                        nc.tensor.transpose(
                            out=trp[:], in_=agg_sb[:, hh * P:(hh + 1) * P],
                            identity=ident_dt[:])
                        mt_sb = wk.tile([P, P], DT, tag="mT2")
                        nc.scalar.activation(mt_sb[:], trp[:], Act.Copy)
                        mts.append(mt_sb)
                    h1T_t = wk.tile([P, H1, P], DT, tag="hTt")
                    for hh in range(H1):
                        nc.sync.dma_start(
                            out=h1T_t[:, hh, :],
                            in_=h1T[hh * P:(hh + 1) * P, t * P:(t + 1) * P])
                    op_ = ps.tile([P, d2], F32, tag="outp")
                    for hh in range(H1):
                        nc.tensor.matmul(out=op_[:], lhsT=mts[hh][:],
                                         rhs=wl1_t[:, hh, :],
                                         start=(hh == 0), stop=False)
                    for hh in range(H1):
                        nc.tensor.matmul(out=op_[:], lhsT=h1T_t[:, hh, :],
                                         rhs=wr1_t[:, hh, :],
                                         start=False, stop=False)
                    nc.tensor.matmul(out=op_[:], lhsT=ones_t[:],
                                     rhs=b1_t[:], start=False, stop=True,
                                     skip_group_check=True)
                    h_sb = wk.tile([P, d2], DT, tag="h_sb")
                    nc.vector.tensor_scalar(out=h_sb[:], in0=op_[:],
                                            scalar1=0.0, scalar2=None,
                                            op0=Alu.max)  # relu + cast
                    hts = []
                    for hh in range(H2):
                        trp = ps.tile([P, P], DT, tag="trp")
                        nc.tensor.transpose(
                            out=trp[:], in_=h_sb[:, hh * P:(hh + 1) * P],
                            identity=ident_dt[:])
                        hT_sb = wk.tile([P, P], DT, tag="hT_sb")
                        nc.vector.tensor_copy(hT_sb[:], trp[:])
                        nc.sync.dma_start(
                            out=h2T[hh * P:(hh + 1) * P, t * P:(t + 1) * P],
                            in_=hT_sb[:])
                        hts.append(hT_sb)
                    prj = ps.tile([P, d3], F32, tag="proj")
                    for hh in range(H2):
                        nc.tensor.matmul(out=prj[:], lhsT=hts[hh][:],
                                         rhs=wl2_t[:, hh, :],
                                         start=(hh == 0), stop=(hh == H2 - 1))
                    prj_sb = wk.tile([P, d3], DT, tag="prj_sb")
                    nc.vector.tensor_copy(prj_sb[:], prj[:])
                    nc.sync.dma_start(out=h2p[t * P:(t + 1) * P, 0:d3],
                                      in_=prj_sb[:])

            nc.gpsimd.collective_compute(
                "AllGather", Alu.bypass,
                replica_groups=[list(range(cfg.NC))],
                ins=[h2p.opt()], outs=[h2pfull.opt()])

            # ---------------- Layer 3 ----------------
            # log_softmax without max-subtraction (logit scale is small):
            # z - ln(sum(exp(z))); Exp accumulates per tile, one Ln at the
            # end (avoids per-tile activation-table reloads)
            se_all = cp.tile([P, cfg.TPC], F32)
            z_all = cp.tile([P, cfg.TPC, d3], F32)
            for gi, tl in enumerate(plan.groups):
                m_t, s_t = gathers(gi, h2pfull, cfg.EL3)
                gb = plan.gbase[gi]
                for t in tl:
                    op_ = ps.tile([P, d3], F32, tag="proj")
                    chunks = tile_chunks(t)
                    for ci, c in enumerate(chunks):
                        nc.tensor.matmul(
                            out=op_[:], lhsT=s_t[:, c - gb, :],
                            rhs=m_t[:, c - gb, 0:d3],
                            start=(ci == 0), stop=False)
                    h2T_t = wk.tile([P, H2, P], DT, tag="hTt")
                    for hh in range(H2):
                        nc.sync.dma_start(
                            out=h2T_t[:, hh, :],
                            in_=h2T[hh * P:(hh + 1) * P, t * P:(t + 1) * P])
                    for hh in range(H2):
                        nc.tensor.matmul(out=op_[:], lhsT=h2T_t[:, hh, :],
                                         rhs=wr2_t[:, hh, :],
                                         start=False, stop=False,
                                         skip_group_check=True)
                    nc.tensor.matmul(out=op_[:], lhsT=ones_t[:],
                                     rhs=b2_t[:], start=False, stop=True,
                                     skip_group_check=True)
                    nc.vector.tensor_copy(z_all[:, t, :], op_[:])
                    e_dummy = wk.tile([P, d3], F32, tag="e_sb")
                    nc.scalar.activation(e_dummy[:], op_[:], Act.Exp,
                                         accum_out=se_all[:, t:t + 1])
            ls_all = cp.tile([P, cfg.TPC], F32)
            nc.scalar.activation(ls_all[:], se_all[:], Act.Ln)
            for t in range(cfg.TPC):
                out_sb = wk.tile([P, d3], F32, tag="out_sb")
                nc.vector.tensor_scalar(out=out_sb[:], in0=z_all[:, t, :],
                                        scalar1=ls_all[:, t:t + 1],
                                        scalar2=None, op0=Alu.subtract)
                nc.sync.dma_start(out=out_t[t * P:(t + 1) * P, :],
                                  in_=out_sb[:])

    nc.compile()
    return nc


_NC_CACHE = {}


def get_nc(cfg, plan):
    key = (cfg.key(), plan.NAB.tobytes())
    if key not in _NC_CACHE:
        _NC_CACHE[key] = build_nc(cfg, plan)
    return _NC_CACHE[key]


def run(cfg, inputs, trace=False, tmpdir=None):
    x = np.asarray(inputs["x"], np.float32)
    plan, idx_full, s_arr = host_prep(
        cfg, x, np.asarray(inputs["edge_index"]),
        np.asarray(inputs["edge_attr"], np.float32))
    d0, d1, d2, d3 = cfg.D
    H1, H2 = d1 // P, d2 // P
    npDT = np_bf16 if cfg.bf16 else np.float32

    xpad = np.zeros((cfg.NPAD, d0), np.float32)
    xpad[:cfg.N] = x
    xpad = xpad.astype(npDT)
    Wl1 = np.asarray(inputs["Wl1"], np.float32)
    Wr1 = np.asarray(inputs["Wr1"], np.float32)
    Wl2 = np.asarray(inputs["Wl2"], np.float32)
    Wr2 = np.asarray(inputs["Wr2"], np.float32)
    shared = {
        "xfull": xpad,
        "wl0": np.asarray(inputs["Wl0"], np.float32).astype(npDT),
        "wr0": np.asarray(inputs["Wr0"], np.float32).astype(npDT),
        "wl1": Wl1.reshape(H1, P, d2).transpose(1, 0, 2).astype(npDT),
        "wr1": Wr1.reshape(H1, P, d2).transpose(1, 0, 2).astype(npDT),
        "wl2": Wl2.reshape(H2, P, d3).transpose(1, 0, 2).astype(npDT),
        "wr2": Wr2.reshape(H2, P, d3).transpose(1, 0, 2).astype(npDT),
        "b0": (np.asarray(inputs["bl0"]) + np.asarray(inputs["br0"]))
        .astype(np.float32)[None, :].astype(npDT),
        "b1": (np.asarray(inputs["bl1"]) + np.asarray(inputs["br1"]))
        .astype(np.float32)[None, :].astype(npDT),
        "b2": (np.asarray(inputs["bl2"]) + np.asarray(inputs["br2"]))
        .astype(np.float32)[None, :].astype(npDT),
    }
    in_maps = []
    for k in range(cfg.NC):
        in_maps.append({
            **shared,
            "xT": np.ascontiguousarray(
                xpad[k * cfg.SHARD:(k + 1) * cfg.SHARD].T),
            "idx": idx_full[k],
            "sfull": s_arr[k].astype(npDT),
        })
    nc = get_nc(cfg, plan)
    res = run_bass_kernel_spmd(nc, in_maps, core_ids=list(range(cfg.NC)),
                               trace=trace, tmpdir=tmpdir)
    out = np.concatenate([res.results[k]["out"] for k in range(cfg.NC)],
                         axis=0)[:cfg.N]
    return np.ascontiguousarray(out.astype(np.float32)), res


def kernel(**inputs):
    cfg = Cfg()
    out, _ = run(cfg, inputs)
    return out
